# revision 25
# baseline (speedup 1.0000x reference)
"""Trainium2 Bass kernel for nn_DenseModel_51926154609008 (weighted-rank
contrastive CE loss) — fp8 DoubleRow edition.

Math (reference semantics, no sort needed):
  scores = q @ p.T                       [B=2048, P=16384]
  t_i    = scores[i, 8*i]                (positive/target score, exact fp32
                                          on host)
  rank_i = #{j : scores[i, j] > t_i}     (argsort position == exceed count)
  lse_i  = logsumexp(scores[i, :])
  loss   = mean((lse_i - t_i) * (1 + 2.6*exp(-(rank_i-1)^2 / (2*1.8^2))))

Sharding: passage-parallel (P split across 8 cores, q replicated).

fp8 strategy: q, p quantized host-side to e4m3 (ml_dtypes.float8_e4m3).
PE runs MatmulPerfMode.DoubleRow (2 fp8 k-chunks of 128 per instruction,
0.5 cycles per moving row = 157 TF/s, 2x the bf16 rate), so each
[128q x 512p] PSUM bank takes 3 matmuls instead of 6 (192 total, 216 ns
steady spacing measured).  Score error std ~1.04 (scores' std is 27.7);
host-emulated loss rel err 3.4e-4, far under the 2e-2 gate.  Ranks only
matter for rank<=~8 queries (the Gaussian weight dies by rank 10) whose
top-score gaps are >> the fp8 noise.

m-major consumer structure (one [128, 2048] 4-bank PSUM tile per query
m-tile, double buffered over the 8 banks): the PE fills a tile with 12
DoubleRow matmuls (2.59 us); one 2048-wide ACTIVATE Exp (2.2 us
including the fused accumulator read) produces the per-query slab
sumexp AND a bf16 junk exp tile je in SBUF; one 2048-wide DVE count
(2.3 us) compares je > theta_i = exp(t_i - C) (exp is monotone; bf16
rounding only flips |s - t| <~ 2^-9 which is noise vs the fp8 error).
Every engine's per-m-tile cost sits under the PE's 2.59 us, so the
kernel is PE-bound at the fp8 roofline.  PSUM banks are released by the
ACT alone (the count reads je from SBUF, not PSUM).

Self-column masking: inputs are rotated per-core so own queries land at
m-tiles 8, 9; those two counts use scalar_tensor_tensor with a bf16 0/1
mask ([128,2048], zero at (r, 8r) resp. (r, 1024+8r)).

theta underflow (t_i < ~40 -> theta ~ 0 in fp32/bf16) only mis-counts
queries whose true rank is already in the hundreds+, where the Gaussian
weight is exactly 1 either way.

DMA: issue cost is ~0.65 us per dma_start on the issuing sequencer, so
the first m-tile's operands are spread over four sequencers (Sync +
Scalar: p8, GpSimd: q8 first columns + masks, Vector: q8 bulk).

Host combines per-m-tile partials ([128, 16] sumexp + count tiles per
core) and evaluates the tiny [2048] tail in fp64.
"""

import sys

import numpy as np

sys.path.insert(0, "/opt/trn_rl_repo")

import concourse.bacc as bacc  # noqa: E402
import concourse.bass as bass  # noqa: E402
import concourse.mybir as mybir  # noqa: E402
import concourse.tile as tile  # noqa: E402
from concourse.bass_utils import run_bass_kernel_spmd  # noqa: E402

# Problem shape (hardcoded per the task contract).
B = 2048
D = 768
NP = 8
P = B * NP  # 16384
NCORES = 8
PSLAB = P // NCORES  # 2048 passage columns per core
KCH = D // 128  # 6 contraction chunks
KP = KCH // 2  # 3 DoubleRow chunk-pairs
MT = B // 128  # 16 query m-tiles
QSLAB = B // NCORES  # 256 queries owned per core
OWN_M = 8  # own queries sit at m-tiles 8,9

C_SHIFT = 128.0  # fixed exp shift: exp(s - C) never overflows

ALPHA = 2.6
OPTIMAL_RANK = 1.0
SIGMA = 1.8

_STATE: dict = {}


def _build_nc():
    nc = bacc.Bacc("TRN2", target_bir_lowering=False, debug=False,
                   num_devices=NCORES)

    f32 = mybir.dt.float32
    bf16 = mybir.dt.bfloat16
    fp8 = mybir.dt.float8e4

    # DRAM layout: [KP, 2, 128, cols] so pair c plane kk DMAs to
    # q8p[c][:, kk, :] as a plain 2-D [128, cols] transfer.
    qT_d = nc.dram_tensor("qT", [KP, 2, 128, B], fp8,
                          kind="ExternalInput").ap()
    pT_d = nc.dram_tensor("pT", [KP, 2, 128, PSLAB], fp8,
                          kind="ExternalInput").ap()
    th_d = nc.dram_tensor("thv", [128, MT], f32, kind="ExternalInput").ap()
    # stats output: cols 0:MT+1 = per-m-tile sumexp, MT+1:2MT+2 = counts
    st_d = nc.dram_tensor("st_out", [128, 2 * MT + 2], f32,
                          kind="ExternalOutput").ap()

    with tile.TileContext(nc) as tc:
        with (
            tc.tile_pool(name="weights", bufs=1) as wpool,
            tc.tile_pool(name="stats", bufs=1) as spool,
            tc.tile_pool(name="je", bufs=4) as jepool,
            tc.tile_pool(name="psum", bufs=2,
                         space=bass.MemorySpace.PSUM) as ppool,
        ):
            # per-pair operand tiles: keeps the DoubleRow [128, 2, cols]
            # reads inside one tile so the scheduler's flat byte-range
            # dependency tracking doesn't chain them to unrelated loads
            q8p = [wpool.tile([128, 2, B], fp8, name=f"q8p{c}", tag=f"q8p{c}")
                   for c in range(KP)]
            p8p = [wpool.tile([128, 2, PSLAB], fp8, name=f"p8p{c}",
                              tag=f"p8p{c}") for c in range(KP)]
            thv = spool.tile([128, MT], f32, name="thv", tag="thv")
            # one wide mask W[r, c] = (c - 8r - 1024 != 0), so
            # msk9 = W[:, 0:2048] (zero at 1024+8r) and
            # msk8 = W[:, 1024:3072] (zero at 8r within the slice)
            mskw = spool.tile([128, PSLAB + 1024], bf16, name="mskw",
                              tag="mskw")
            it16 = spool.tile([128, PSLAB + 1024], mybir.dt.int16,
                              name="it16", tag="it16")
            st_sb = spool.tile([128, 2 * MT + 2], f32, name="st_sb",
                               tag="st_sb")
            negc = spool.tile([128, 1], f32, name="negc", tag="negc")

            # --- input DMA schedule (all plain 2-D [128, cols] full-plane
            # chunks; mixed-rank patterns scramble data).  m-tile 0 needs
            # all of q8 cols 0:128 and ALL of p8, so p8 pairs 0-1 go on
            # Sync, pair 2 on Scalar, q8 planes on GpSimd.
            nc.vector.memset(negc[:], -C_SHIFT)
            for c in range(KP):
                nc.gpsimd.dma_start(q8p[c][:, 0, :], qT_d[c, 0])
                nc.gpsimd.dma_start(q8p[c][:, 1, :], qT_d[c, 1])
            for c in range(2):
                for kk in range(2):
                    nc.sync.dma_start(p8p[c][:, kk, :], pT_d[c, kk])
            nc.scalar.dma_start(p8p[2][:, 0, :], pT_d[2, 0])
            nc.scalar.dma_start(p8p[2][:, 1, :], pT_d[2, 1])
            nc.scalar.dma_start(thv[:], th_d[:])
            # masks generated on-device (saves 1MB of ramp DMA traffic)
            nc.gpsimd.iota(it16[:], [[1, PSLAB + 1024]], base=-1024,
                           channel_multiplier=-8)
            nc.vector.tensor_scalar(mskw[:], it16[:], 0, None,
                                    op0=mybir.AluOpType.not_equal)

            dr = mybir.MatmulPerfMode.DoubleRow

            def consume(m, lo, hi, col):
                """Exp+sumexp (Scalar) and rank count (Vector) for
                ps[:, lo:hi] of m-tile m, accumulating into stats col."""
                sl = slice(lo, hi)
                nc.scalar.activation(
                    je[:, sl], ps[:, sl], mybir.ActivationFunctionType.Exp,
                    bias=negc[:], scale=1.0,
                    accum_out=st_sb[:, col:col + 1],
                )
                ccol = MT + 1 + col
                # the count overwrites je in place: je is dead after it
                # (sumexp comes from the ACT accumulator), and dropping
                # the separate junk tile removes a pool + sync edges
                if m in (OWN_M, OWN_M + 1):
                    off = 1024 if m == OWN_M else 0
                    nc.vector.scalar_tensor_tensor(
                        out=je[:, sl], in0=je[:, sl],
                        scalar=thv[:, m:m + 1],
                        in1=mskw[:, off + lo:off + hi],
                        op0=mybir.AluOpType.is_gt,
                        op1=mybir.AluOpType.mult,
                        accum_out=st_sb[:, ccol:ccol + 1],
                    )
                else:
                    nc.vector.tensor_scalar(
                        je[:, sl], je[:, sl], thv[:, m:m + 1], None,
                        op0=mybir.AluOpType.is_gt,
                        op1=mybir.AluOpType.add,
                        accum_out=st_sb[:, ccol:ccol + 1],
                    )

            def mm(ps, m, b, c):
                nc.tensor.matmul(
                    ps[:, b * 512:(b + 1) * 512],
                    q8p[c][:, :, m * 128:(m + 1) * 128],
                    p8p[c][:, :, b * 512:(b + 1) * 512],
                    start=(c == 0),
                    stop=(c == KP - 1),
                    perf_mode=dr,
                )

            # m-tiles 0,1 run plane-major with PLAIN single-plane fp8
            # matmuls (1 cyc/row, same 216 ns per MM as DoubleRow but only
            # ONE operand plane per instruction): the first 8 MMs need only
            # the first plane on each DMA ring, so the PE starts ~2.6 us
            # earlier and does its p-state warmup during wire-idle time.
            ps01 = [ppool.tile([128, PSLAB], f32, name="ps", tag="ps")
                    for _ in range(2)]
            for c in range(KP):
                for kk in range(2):
                    for m in range(2):
                        for b in range(4):
                            nc.tensor.matmul(
                                ps01[m][:, b * 512:(b + 1) * 512],
                                q8p[c][:, kk, m * 128:(m + 1) * 128],
                                p8p[c][:, kk, b * 512:(b + 1) * 512],
                                start=(c == 0 and kk == 0),
                                stop=(c == KP - 1 and kk == 1),
                            )
            for m in range(2):
                ps = ps01[m]
                je = jepool.tile([128, PSLAB], bf16, name="je", tag="je")
                consume(m, 0, 2048, m)

            for m in range(2, MT):
                ps = ppool.tile([128, PSLAB], f32, name="ps", tag="ps")
                for b in range(4):
                    for c in range(KP):
                        mm(ps, m, b, c)
                je = jepool.tile([128, PSLAB], bf16, name="je", tag="je")
                if m == MT - 1:
                    # split the last tile's consumers so they overlap the
                    # final matmuls instead of serializing after them
                    consume(m, 0, 1024, m)
                    consume(m, 1024, 2048, m + 1)
                else:
                    consume(m, 0, 2048, m)

            nc.sync.dma_start(st_d[:], st_sb[:])

    nc.compile()
    return nc


def _perm(c):
    """Rotation putting core c's own queries at m-tiles OWN_M, OWN_M+1."""
    return np.roll(np.arange(B), OWN_M * 128 - c * QSLAB)


def prepare(q, p):
    """Host-side shard prep. Returns (in_maps, t32, perms)."""
    import ml_dtypes
    fp8 = ml_dtypes.float8_e4m3
    q = np.ascontiguousarray(np.asarray(q, dtype=np.float32))
    p = np.ascontiguousarray(np.asarray(p, dtype=np.float32))

    # target scores t_i = q_i . p_{8i} (exact fp32; threshold + host tail)
    t32 = np.einsum("ij,ij->i", q, p[::NP], dtype=np.float64).astype(np.float32)
    # count threshold in exp space: theta_i = exp(t_i - C); underflow to 0
    # only affects queries whose rank is huge (weight exactly 1) either way
    th32 = np.exp(t32.astype(np.float64) - C_SHIFT).astype(np.float32)

    q8 = q.astype(fp8)  # [B, D]
    p8 = p.astype(fp8)  # [P, D]
    # DRAM layout [KP, 2, 128, cols]: pair c plane kk holds rows
    # (2c+kk)*128 .. +127 of the transposed [D, cols] operand
    qT8 = np.ascontiguousarray(q8.T.reshape(KP, 2, 128, B))

    in_maps = []
    perms = []
    for c in range(NCORES):
        perm = _perm(c)
        perms.append(perm)
        qTc = np.ascontiguousarray(qT8[:, :, :, perm])
        pTc = np.ascontiguousarray(
            p8[c * PSLAB:(c + 1) * PSLAB].T.reshape(KP, 2, 128, PSLAB))
        thc = np.ascontiguousarray(th32[perm].reshape(MT, 128).T)
        in_maps.append({"qT": qTc, "pT": pTc, "thv": thc})
    return in_maps, t32, perms


def finalize(results, t32, perms):
    """Combine per-core partials into the scalar loss (fp64 host tail)."""
    se_tot = np.zeros(B, dtype=np.float64)
    cnt_tot = np.zeros(B, dtype=np.float64)
    for c in range(NCORES):
        perm = perms[c]
        # col m, row r -> query pi = m*128 + r; the last m-tile's stats
        # are split across cols MT-1 and MT (half-slab each)
        st = results[c]["st_out"].astype(np.float64)
        se, cnt = st[:, :MT + 1], st[:, MT + 1:]
        se[:, MT - 1] += se[:, MT]
        cnt[:, MT - 1] += cnt[:, MT]
        se_tot[perm] += se[:, :MT].T.ravel()
        cnt_tot[perm] += cnt[:, :MT].T.ravel()
    lse = C_SHIFT + np.log(se_tot)
    raw = lse - t32.astype(np.float64)
    w = 1.0 + ALPHA * np.exp(-((cnt_tot - OPTIMAL_RANK) ** 2)
                             / (2.0 * SIGMA ** 2))
    return np.float32(np.mean(raw * w))


def _get_nc():
    if "nc" not in _STATE:
        _STATE["nc"] = _build_nc()
    return _STATE["nc"]


def kernel(q_reps, p_reps, n_passages):
    assert int(np.asarray(n_passages)) == NP
    nc = _get_nc()
    in_maps, t32, perms = prepare(q_reps, p_reps)
    # rare transient NRT_EXEC_UNIT_UNRECOVERABLE: reset the PJRT client
    # and retry with backoff
    import time
    last = None
    for attempt in range(4):
        try:
            res = run_bass_kernel_spmd(nc, in_maps,
                                       core_ids=list(range(NCORES)))
            return finalize(res.results, t32, perms)
        except Exception as e:
            last = e
            try:
                import jax
                jax.clear_caches()
                jax.extend.backend.clear_backends()
            except Exception:
                pass
            time.sleep(10 * (attempt + 1))
    raise last


def run_profiled(q_reps, p_reps, n_passages, trace=True):
    """Same as kernel() but returns (loss, BassKernelResults) with NTFF
    profile (requires the antenv.axon_hooks shim; see _install_ntff_shim)."""
    nc = _get_nc()
    in_maps, t32, perms = prepare(q_reps, p_reps)
    res = run_bass_kernel_spmd(nc, in_maps, core_ids=list(range(NCORES)),
                               trace=trace)
    loss = finalize(res.results, t32, perms)
    return loss, res


def _install_ntff_shim():
    """Provide antenv.axon_hooks (absent in this image) so trace=True works."""
    import types
    import antenv
    if "antenv.axon_hooks" in sys.modules:
        return
    mod = types.ModuleType("antenv.axon_hooks")
    mod._hook = None
    mod.set_axon_ntff_profile_hook = lambda h: setattr(mod, "_hook", h)
    mod.get_axon_ntff_profile_hook = lambda: mod._hook
    sys.modules["antenv.axon_hooks"] = mod
    antenv.axon_hooks = mod
    try:
        from trn_agent_boot.trn_boot import _ntff_profile_via_ctypes
        hook = _ntff_profile_via_ctypes("/opt/axon/libaxon_pjrt.so")
        if hook is not None:
            mod._hook = hook
    except Exception:
        pass


# revision 27
# speedup vs baseline: 1.0260x; 1.0260x over previous
"""Trainium2 Bass kernel for nn_DenseModel_51926154609008 (weighted-rank
contrastive CE loss) — fp8 DoubleRow edition.

Math (reference semantics, no sort needed):
  scores = q @ p.T                       [B=2048, P=16384]
  t_i    = scores[i, 8*i]                (positive/target score, exact fp32
                                          on host)
  rank_i = #{j : scores[i, j] > t_i}     (argsort position == exceed count)
  lse_i  = logsumexp(scores[i, :])
  loss   = mean((lse_i - t_i) * (1 + 2.6*exp(-(rank_i-1)^2 / (2*1.8^2))))

Sharding: passage-parallel (P split across 8 cores, q replicated).

fp8 strategy: q, p quantized host-side to e4m3 (ml_dtypes.float8_e4m3).
PE runs MatmulPerfMode.DoubleRow (2 fp8 k-chunks of 128 per instruction,
0.5 cycles per moving row = 157 TF/s, 2x the bf16 rate), so each
[128q x 512p] PSUM bank takes 3 matmuls instead of 6 (192 total, 216 ns
steady spacing measured).  Score error std ~1.04 (scores' std is 27.7);
host-emulated loss rel err 3.4e-4, far under the 2e-2 gate.  Ranks only
matter for rank<=~8 queries (the Gaussian weight dies by rank 10) whose
top-score gaps are >> the fp8 noise.

m-major consumer structure (one [128, 2048] 4-bank PSUM tile per query
m-tile, double buffered over the 8 banks): the PE fills a tile with 12
DoubleRow matmuls (2.59 us); one 2048-wide ACTIVATE Exp (2.2 us
including the fused accumulator read) produces the per-query slab
sumexp AND a bf16 junk exp tile je in SBUF; one 2048-wide DVE count
(2.3 us) compares je > theta_i = exp(t_i - C) (exp is monotone; bf16
rounding only flips |s - t| <~ 2^-9 which is noise vs the fp8 error).
Every engine's per-m-tile cost sits under the PE's 2.59 us, so the
kernel is PE-bound at the fp8 roofline.  PSUM banks are released by the
ACT alone (the count reads je from SBUF, not PSUM).

Self-column masking: inputs are rotated per-core so own queries land at
m-tiles 8, 9; those two counts use scalar_tensor_tensor with a bf16 0/1
mask ([128,2048], zero at (r, 8r) resp. (r, 1024+8r)).

theta underflow (t_i < ~40 -> theta ~ 0 in fp32/bf16) only mis-counts
queries whose true rank is already in the hundreds+, where the Gaussian
weight is exactly 1 either way.

DMA: issue cost is ~0.65 us per dma_start on the issuing sequencer, so
the first m-tile's operands are spread over four sequencers (Sync +
Scalar: p8, GpSimd: q8 first columns + masks, Vector: q8 bulk).

Host combines per-m-tile partials ([128, 16] sumexp + count tiles per
core) and evaluates the tiny [2048] tail in fp64.
"""

import sys

import numpy as np

sys.path.insert(0, "/opt/trn_rl_repo")

import concourse.bacc as bacc  # noqa: E402
import concourse.bass as bass  # noqa: E402
import concourse.mybir as mybir  # noqa: E402
import concourse.tile as tile  # noqa: E402
from concourse.bass_utils import run_bass_kernel_spmd  # noqa: E402

# Problem shape (hardcoded per the task contract).
B = 2048
D = 768
NP = 8
P = B * NP  # 16384
NCORES = 8
PSLAB = P // NCORES  # 2048 passage columns per core
KCH = D // 128  # 6 contraction chunks
KP = KCH // 2  # 3 DoubleRow chunk-pairs
MT = B // 128  # 16 query m-tiles
QSLAB = B // NCORES  # 256 queries owned per core
OWN_M = 8  # own queries sit at m-tiles 8,9

C_SHIFT = 128.0  # fixed exp shift: exp(s - C) never overflows

ALPHA = 2.6
OPTIMAL_RANK = 1.0
SIGMA = 1.8

_STATE: dict = {}


def _build_nc():
    nc = bacc.Bacc("TRN2", target_bir_lowering=False, debug=False,
                   num_devices=NCORES)

    f32 = mybir.dt.float32
    bf16 = mybir.dt.bfloat16
    fp8 = mybir.dt.float8e4

    # DRAM layout: [KP, 2, 128, cols] so pair c plane kk DMAs to
    # q8p[c][:, kk, :] as a plain 2-D [128, cols] transfer.
    qT_d = nc.dram_tensor("qT", [KP, 2, 128, B], fp8,
                          kind="ExternalInput").ap()
    pT_d = nc.dram_tensor("pT", [KP, 2, 128, PSLAB], fp8,
                          kind="ExternalInput").ap()
    th_d = nc.dram_tensor("thv", [128, MT], f32, kind="ExternalInput").ap()
    # stats output: cols 0:MT+1 = per-m-tile sumexp, MT+1:2MT+2 = counts
    st_d = nc.dram_tensor("st_out", [128, 2 * MT + 2], f32,
                          kind="ExternalOutput").ap()

    with tile.TileContext(nc) as tc:
        with (
            tc.tile_pool(name="weights", bufs=1) as wpool,
            tc.tile_pool(name="stats", bufs=1) as spool,
            tc.tile_pool(name="je", bufs=4) as jepool,
            tc.tile_pool(name="psum", bufs=2,
                         space=bass.MemorySpace.PSUM) as ppool,
        ):
            # per-pair operand tiles: keeps the DoubleRow [128, 2, cols]
            # reads inside one tile so the scheduler's flat byte-range
            # dependency tracking doesn't chain them to unrelated loads
            q8p = [wpool.tile([128, 2, B], fp8, name=f"q8p{c}", tag=f"q8p{c}")
                   for c in range(KP)]
            p8p = [wpool.tile([128, 2, PSLAB], fp8, name=f"p8p{c}",
                              tag=f"p8p{c}") for c in range(KP)]
            thv = spool.tile([128, MT], f32, name="thv", tag="thv")
            # one wide mask W[r, c] = (c - 8r - 1024 != 0), so
            # msk9 = W[:, 0:2048] (zero at 1024+8r) and
            # msk8 = W[:, 1024:3072] (zero at 8r within the slice)
            mskw = spool.tile([128, PSLAB + 1024], bf16, name="mskw",
                              tag="mskw")
            it16 = spool.tile([128, PSLAB + 1024], mybir.dt.int16,
                              name="it16", tag="it16")
            st_sb = spool.tile([128, 2 * MT + 2], f32, name="st_sb",
                               tag="st_sb")
            negc = spool.tile([128, 1], f32, name="negc", tag="negc")

            # --- input DMA schedule (all plain 2-D [128, cols] full-plane
            # chunks; mixed-rank patterns scramble data).  m-tile 0 needs
            # all of q8 cols 0:128 and ALL of p8, so p8 pairs 0-1 go on
            # Sync, pair 2 on Scalar, q8 planes on GpSimd.
            nc.vector.memset(negc[:], -C_SHIFT)
            for c in range(KP):
                nc.gpsimd.dma_start(q8p[c][:, 0, :], qT_d[c, 0])
                nc.gpsimd.dma_start(q8p[c][:, 1, :], qT_d[c, 1])
            for c in range(2):
                for kk in range(2):
                    nc.sync.dma_start(p8p[c][:, kk, :], pT_d[c, kk])
            nc.scalar.dma_start(p8p[2][:, 0, :], pT_d[2, 0])
            nc.scalar.dma_start(p8p[2][:, 1, :], pT_d[2, 1])
            nc.scalar.dma_start(thv[:], th_d[:])
            # masks generated on-device (saves 1MB of ramp DMA traffic)
            nc.gpsimd.iota(it16[:], [[1, PSLAB + 1024]], base=-1024,
                           channel_multiplier=-8)
            nc.vector.tensor_scalar(mskw[:], it16[:], 0, None,
                                    op0=mybir.AluOpType.not_equal)

            dr = mybir.MatmulPerfMode.DoubleRow

            def consume(m, lo, hi, col):
                """Exp+sumexp (Scalar) and rank count (Vector) for
                ps[:, lo:hi] of m-tile m, accumulating into stats col."""
                sl = slice(lo, hi)
                nc.scalar.activation(
                    je[:, sl], ps[:, sl], mybir.ActivationFunctionType.Exp,
                    bias=negc[:], scale=1.0,
                    accum_out=st_sb[:, col:col + 1],
                )
                ccol = MT + 1 + col
                # the count overwrites je in place: je is dead after it
                # (sumexp comes from the ACT accumulator), and dropping
                # the separate junk tile removes a pool + sync edges
                if m in (OWN_M, OWN_M + 1):
                    off = 1024 if m == OWN_M else 0
                    nc.vector.scalar_tensor_tensor(
                        out=je[:, sl], in0=je[:, sl],
                        scalar=thv[:, m:m + 1],
                        in1=mskw[:, off + lo:off + hi],
                        op0=mybir.AluOpType.is_gt,
                        op1=mybir.AluOpType.mult,
                        accum_out=st_sb[:, ccol:ccol + 1],
                    )
                else:
                    nc.vector.tensor_scalar(
                        je[:, sl], je[:, sl], thv[:, m:m + 1], None,
                        op0=mybir.AluOpType.is_gt,
                        op1=mybir.AluOpType.add,
                        accum_out=st_sb[:, ccol:ccol + 1],
                    )

            def mm(ps, m, b, c):
                nc.tensor.matmul(
                    ps[:, b * 512:(b + 1) * 512],
                    q8p[c][:, :, m * 128:(m + 1) * 128],
                    p8p[c][:, :, b * 512:(b + 1) * 512],
                    start=(c == 0),
                    stop=(c == KP - 1),
                    perf_mode=dr,
                )

            # m-tiles 0,1 run pair-major (c outer) so the PE starts as soon
            # as operand pair 0 lands and overlaps the rest of the input
            # DMA (and its own p-state warmup) with real work.
            ps01 = [ppool.tile([128, PSLAB], f32, name="ps", tag="ps")
                    for _ in range(2)]
            for c in range(KP):
                for m in range(2):
                    for b in range(4):
                        mm(ps01[m], m, b, c)
            for m in range(2):
                ps = ps01[m]
                je = jepool.tile([128, PSLAB], bf16, name="je", tag="je")
                consume(m, 0, 2048, m)

            for m in range(2, MT):
                ps = ppool.tile([128, PSLAB], f32, name="ps", tag="ps")
                # last tile: fill banks high-to-low and consume half 1
                # first, so its ACT+count overlap the final matmuls and
                # only the half-0 chain trails the last MM
                border = (3, 2, 1, 0) if m == MT - 1 else range(4)
                for b in border:
                    for c in range(KP):
                        mm(ps, m, b, c)
                je = jepool.tile([128, PSLAB], bf16, name="je", tag="je")
                if m == MT - 1:
                    consume(m, 1024, 2048, m + 1)
                    consume(m, 0, 1024, m)
                else:
                    consume(m, 0, 2048, m)

            nc.sync.dma_start(st_d[:], st_sb[:])

    nc.compile()
    return nc


def _perm(c):
    """Rotation putting core c's own queries at m-tiles OWN_M, OWN_M+1."""
    return np.roll(np.arange(B), OWN_M * 128 - c * QSLAB)


def prepare(q, p):
    """Host-side shard prep. Returns (in_maps, t32, perms)."""
    import ml_dtypes
    fp8 = ml_dtypes.float8_e4m3
    q = np.ascontiguousarray(np.asarray(q, dtype=np.float32))
    p = np.ascontiguousarray(np.asarray(p, dtype=np.float32))

    # target scores t_i = q_i . p_{8i} (exact fp32; threshold + host tail)
    t32 = np.einsum("ij,ij->i", q, p[::NP], dtype=np.float64).astype(np.float32)
    # count threshold in exp space: theta_i = exp(t_i - C); underflow to 0
    # only affects queries whose rank is huge (weight exactly 1) either way
    th32 = np.exp(t32.astype(np.float64) - C_SHIFT).astype(np.float32)

    q8 = q.astype(fp8)  # [B, D]
    p8 = p.astype(fp8)  # [P, D]
    # DRAM layout [KP, 2, 128, cols]: pair c plane kk holds rows
    # (2c+kk)*128 .. +127 of the transposed [D, cols] operand
    qT8 = np.ascontiguousarray(q8.T.reshape(KP, 2, 128, B))

    in_maps = []
    perms = []
    for c in range(NCORES):
        perm = _perm(c)
        perms.append(perm)
        qTc = np.ascontiguousarray(qT8[:, :, :, perm])
        pTc = np.ascontiguousarray(
            p8[c * PSLAB:(c + 1) * PSLAB].T.reshape(KP, 2, 128, PSLAB))
        thc = np.ascontiguousarray(th32[perm].reshape(MT, 128).T)
        in_maps.append({"qT": qTc, "pT": pTc, "thv": thc})
    return in_maps, t32, perms


def finalize(results, t32, perms):
    """Combine per-core partials into the scalar loss (fp64 host tail)."""
    se_tot = np.zeros(B, dtype=np.float64)
    cnt_tot = np.zeros(B, dtype=np.float64)
    for c in range(NCORES):
        perm = perms[c]
        # col m, row r -> query pi = m*128 + r; the last m-tile's stats
        # are split across cols MT-1 and MT (half-slab each)
        st = results[c]["st_out"].astype(np.float64)
        se, cnt = st[:, :MT + 1], st[:, MT + 1:]
        se[:, MT - 1] += se[:, MT]
        cnt[:, MT - 1] += cnt[:, MT]
        se_tot[perm] += se[:, :MT].T.ravel()
        cnt_tot[perm] += cnt[:, :MT].T.ravel()
    lse = C_SHIFT + np.log(se_tot)
    raw = lse - t32.astype(np.float64)
    w = 1.0 + ALPHA * np.exp(-((cnt_tot - OPTIMAL_RANK) ** 2)
                             / (2.0 * SIGMA ** 2))
    return np.float32(np.mean(raw * w))


def _get_nc():
    if "nc" not in _STATE:
        _STATE["nc"] = _build_nc()
    return _STATE["nc"]


def kernel(q_reps, p_reps, n_passages):
    assert int(np.asarray(n_passages)) == NP
    nc = _get_nc()
    in_maps, t32, perms = prepare(q_reps, p_reps)
    # rare transient NRT_EXEC_UNIT_UNRECOVERABLE: reset the PJRT client
    # and retry with backoff
    import time
    last = None
    for attempt in range(4):
        try:
            res = run_bass_kernel_spmd(nc, in_maps,
                                       core_ids=list(range(NCORES)))
            return finalize(res.results, t32, perms)
        except Exception as e:
            last = e
            try:
                import jax
                jax.clear_caches()
                jax.extend.backend.clear_backends()
            except Exception:
                pass
            time.sleep(10 * (attempt + 1))
    raise last


def run_profiled(q_reps, p_reps, n_passages, trace=True):
    """Same as kernel() but returns (loss, BassKernelResults) with NTFF
    profile (requires the antenv.axon_hooks shim; see _install_ntff_shim)."""
    nc = _get_nc()
    in_maps, t32, perms = prepare(q_reps, p_reps)
    res = run_bass_kernel_spmd(nc, in_maps, core_ids=list(range(NCORES)),
                               trace=trace)
    loss = finalize(res.results, t32, perms)
    return loss, res


def _install_ntff_shim():
    """Provide antenv.axon_hooks (absent in this image) so trace=True works."""
    import types
    import antenv
    if "antenv.axon_hooks" in sys.modules:
        return
    mod = types.ModuleType("antenv.axon_hooks")
    mod._hook = None
    mod.set_axon_ntff_profile_hook = lambda h: setattr(mod, "_hook", h)
    mod.get_axon_ntff_profile_hook = lambda: mod._hook
    sys.modules["antenv.axon_hooks"] = mod
    antenv.axon_hooks = mod
    try:
        from trn_agent_boot.trn_boot import _ntff_profile_via_ctypes
        hook = _ntff_profile_via_ctypes("/opt/axon/libaxon_pjrt.so")
        if hook is not None:
            mod._hook = hook
    except Exception:
        pass
